# revision 1
# baseline (speedup 1.0000x reference)
"""Block-sparse linear layer (x @ (mask*W).T + bias) on 8 TRN2 NeuronCores.

Strategy: data-parallel over batch rows. Each core gets 1024 rows of x
(transposed to [k, m] on host, cast to bf16), the packed kept weight
blocks (bf16), and bias. On-device: out.T tile [o=128, m=1024] accumulates
in PSUM over the 16 kept k-subtiles (k-subtile = 128 rows), with W tiles
stationary and x slabs moving. PSUM is evicted through the vector/scalar
engines with the per-partition bias add fused, then DMA'd out. The host
reassembles the full [8192, 4096] fp32 output.
"""

import sys
import types

import numpy as np
import ml_dtypes

BATCH = 8192
SIZE = 4096
NB = 16
BLOCK = 256
NCORES = 8
MC = BATCH // NCORES  # 1024 rows per core
P = 128
KS = SIZE // P  # 32 k-subtiles
OT = SIZE // P  # 32 o-tiles
MM_N = 512  # moving free dim per matmul

_BUILD_CACHE = {}


def _install_ntff_hook():
    # Register the axon NTFF profiling hook if the image's antenv lacks it.
    if "antenv.axon_hooks" in sys.modules:
        return
    try:
        from trn_agent_boot.trn_boot import _ntff_profile_via_ctypes

        hook = _ntff_profile_via_ctypes("/opt/axon/libaxon_pjrt.so")
        mod = types.ModuleType("antenv.axon_hooks")
        mod.get_axon_ntff_profile_hook = lambda: hook
        sys.modules["antenv.axon_hooks"] = mod
    except Exception:
        pass


def _block_keep_from_mask(mask):
    """Return [NB, NB] bool of kept blocks if mask is block-constant, else None."""
    m4 = np.asarray(mask).reshape(NB, BLOCK, NB, BLOCK)
    keep = m4[:, 0, :, 0]
    uniform = np.all(m4 == keep[:, None, :, None])
    return keep if uniform else None


def _ks_lists(keep):
    """Per o-tile (128 outputs) list of kept k-subtile indices, padded to
    a uniform length (padding points at subtile 0 with zero weights)."""
    lists = []
    for t in range(OT):
        i = (t * P) // BLOCK  # o-block row
        ks = []
        for j in range(NB):
            if keep[i, j]:
                base = (j * BLOCK) // P
                ks.extend(range(base, base + BLOCK // P))
        lists.append(ks)
    n_sub = max(1, max(len(l) for l in lists))
    padded = tuple(tuple(l + [-1] * (n_sub - len(l))) for l in lists)
    return padded, n_sub


def _build(ks_lists, n_sub):
    import concourse.mybir as mybir
    import concourse.tile as tile
    from concourse import bacc

    bf16, f32 = mybir.dt.bfloat16, mybir.dt.float32
    nc = bacc.Bacc("TRN2", target_bir_lowering=False)
    xt_d = nc.declare_dram_parameter("xt", [P, KS, MC], bf16, isOutput=False)
    wt_d = nc.declare_dram_parameter("wt", [OT, P, n_sub, P], bf16, isOutput=False)
    bias_d = nc.declare_dram_parameter("biast", [P, OT], f32, isOutput=False)
    out_d = nc.declare_dram_parameter("out", [OT, P, MC], f32, isOutput=True)

    # x DMA issue order: k-subtiles in order of first use across o-tiles.
    ks_order = []
    for t in range(OT):
        for ks in ks_lists[t]:
            if ks >= 0 and ks not in ks_order:
                ks_order.append(ks)
    for ks in range(KS):
        if ks not in ks_order:
            ks_order.append(ks)

    W_POOL_BUFS = 8
    XG = 2  # x chunks per DMA group

    with tile.TileContext(nc) as tc:
        with (
            tc.tile_pool(name="const", bufs=1) as const_pool,
            tc.tile_pool(name="xpool", bufs=1) as xpool,
            tc.tile_pool(name="wpool", bufs=W_POOL_BUFS) as wpool,
            tc.tile_pool(name="opool", bufs=3) as opool,
            tc.tile_pool(name="psum", bufs=4, space="PSUM") as psum_pool,
        ):
            bias_tile = const_pool.tile([P, OT], f32)
            nc.gpsimd.dma_start(out=bias_tile[:], in_=bias_d[:])

            # Warm the PE clock (HAM un-throttles after ~3.4us of sustained
            # matmul activity) with dummy matmuls on zeroed SBUF while the
            # first x/W DMAs are still in flight.
            warm = const_pool.tile([P, MM_N], bf16, name="warm")
            nc.vector.memset(warm[:], 0)
            warm_ps = psum_pool.tile([P, MM_N], f32, name="warm_ps", tag="ps")
            N_WARM = 8
            for i in range(N_WARM):
                nc.tensor.matmul(
                    warm_ps[:],
                    lhsT=warm[:, 0:P],
                    rhs=warm[:],
                    start=(i == 0),
                    stop=(i == N_WARM - 1),
                )

            w_tiles = {}

            def w_alloc(t):
                w_tiles[t] = wpool.tile([P, n_sub, P], bf16, name="w_tile")

            def w_dma(t, engine, half=None):
                if t not in w_tiles:
                    w_alloc(t)
                w = w_tiles[t]
                if half is None:
                    lo, hi = 0, n_sub
                else:
                    step = (n_sub + 1) // 2
                    lo, hi = half * step, min((half + 1) * step, n_sub)
                engine.dma_start(
                    out=w[:, lo:hi, :], in_=wt_d[t, :, lo:hi, :]
                )

            x_ap = {}
            x_groups = []
            for gstart in range(0, len(ks_order), XG):
                grp = ks_order[gstart : gstart + XG]
                lo = min(grp)
                assert grp == list(range(lo, lo + len(grp))), grp
                x_groups.append((lo, len(grp)))

            def x_dma(gi):
                lo, n = x_groups[gi]
                xg = xpool.tile([P, n, MC], bf16, name=f"x_g{gi}", uniquify=False)
                nc.sync.dma_start(out=xg[:], in_=xt_d[:, lo : lo + n, :])
                for off in range(n):
                    x_ap[lo + off] = xg[:, off, :]

            # Startup-critical DMAs on the fast Sync queue, ordered to match
            # PE consumption (the first FOUR o-tiles run interleaved
            # chunk-major, so all four weight-tile first-halves lead).
            # Later W tiles stream on the GpSimd queue.
            for t in range(4):
                w_dma(t, nc.sync, half=0)
            x_dma(0)
            x_dma(1)
            w_dma(2, nc.sync, half=1)
            w_dma(3, nc.sync, half=1)
            x_dma(2)
            x_dma(3)
            w_dma(0, nc.sync, half=1)
            w_dma(1, nc.sync, half=1)
            for gi in range(4, 9):
                x_dma(gi)
            w_dma(4, nc.sync)
            w_dma(5, nc.sync)
            for gi in range(9, len(x_groups)):
                x_dma(gi)

            def emit_block(ts, interleave):
                """Emit the accumulation + eviction for o-tiles `ts`.

                interleave=True: chunk-major across the tiles (each arriving
                x chunk is consumed by every tile that uses it — PE executes
                strictly in order, so this is what absorbs DMA latency).
                interleave=False: tile-major (first tile finishes early so
                its eviction overlaps the next tile's matmuls).
                """
                ps = {t: psum_pool.tile([P, MC], f32, name="ps") for t in ts}
                sets = {t: {ks: s for s, ks in enumerate(ks_lists[t]) if ks >= 0} for t in ts}
                for t in ts:
                    if not sets[t]:  # fully-masked o-tile: zero the PSUM
                        sets[t] = {ks_order[0]: 0}
                n_done = {t: 0 for t in ts}
                if interleave:
                    order = [
                        (c, t)
                        for c in ks_order
                        for t in ts
                        if c in sets[t]
                    ]
                else:
                    order = [(c, t) for t in ts for c in ks_lists[t] if c >= 0]
                for c, t in order:
                    s = sets[t][c]
                    first = n_done[t] == 0
                    n_done[t] += 1
                    last = n_done[t] == len(sets[t])
                    for h in range(MC // MM_N):
                        nc.tensor.matmul(
                            ps[t][:, h * MM_N : (h + 1) * MM_N],
                            lhsT=w_tiles[t][:, s, :],
                            rhs=x_ap[c][:, h * MM_N : (h + 1) * MM_N],
                            start=first,
                            stop=last,
                        )
                    if not interleave and last:
                        _evict(ts, t, ps)
                if interleave:
                    for t in ts:
                        _evict(ts, t, ps)

            def _evict(ts, t, ps):
                # Evict in halves (out-DMA of the first half overlaps the
                # bias-add of the second); even o-tiles on the Vector
                # engine, odd on Scalar, so neighbor evictions parallelize.
                o_tile = opool.tile([P, MC], f32, name="o_tile")
                half = MC // 2
                for h in range(2):
                    sl = slice(h * half, (h + 1) * half)
                    if t % 2 == 0:
                        nc.vector.tensor_scalar_add(
                            o_tile[:, sl], ps[t][:, sl], bias_tile[:, t : t + 1]
                        )
                    else:
                        nc.scalar.add(
                            o_tile[:, sl], ps[t][:, sl], bias_tile[:, t : t + 1]
                        )
                    nc.sync.dma_start(out=out_d[t, :, sl], in_=o_tile[:, sl])

            # First four o-tiles as one interleaved block (their k-chunk
            # sets overlap heavily, maximizing PE work per arriving byte
            # during the x load); middle o-tiles pair-wise; last pair
            # tile-major so the final evictions overlap compute.
            emit_block((0, 1, 2, 3), interleave=True)
            for pair in range(2, OT // 2):
                ts = (2 * pair, 2 * pair + 1)
                for t in ts:
                    if t >= 6:
                        w_dma(t, nc.gpsimd)
                emit_block(ts, interleave=(pair != OT // 2 - 1))
    nc.compile()
    return nc


def _get_kernel(ks_lists, n_sub):
    key = (ks_lists, n_sub)
    if key not in _BUILD_CACHE:
        _BUILD_CACHE[key] = _build(ks_lists, n_sub)
    return _BUILD_CACHE[key]


def kernel(x, weight, bias, mask, _trace=False):
    from concourse.bass_utils import run_bass_kernel_spmd

    _install_ntff_hook()

    x = np.asarray(x)
    weight = np.asarray(weight)
    bias = np.asarray(bias, dtype=np.float32)
    keep = _block_keep_from_mask(mask)
    if keep is None:
        # Mask not block-constant: fall back to a dense schedule with the
        # element-masked weights and every k-subtile kept.
        weight = np.where(np.asarray(mask), weight, 0.0).astype(np.float32)
        keep = np.ones((NB, NB), dtype=bool)
    ks_lists, n_sub = _ks_lists(keep)

    nc = _get_kernel(ks_lists, n_sub)

    # Pack weights: wt[t, p, s, q] = W[t*P + q, ks*P + p] for kept subtile ks.
    w4 = weight.reshape(OT, P, KS, P)  # [t, q, ks, p]
    wt = np.zeros((OT, P, n_sub, P), dtype=ml_dtypes.bfloat16)
    for t in range(OT):
        idx = [ks for ks in ks_lists[t]]
        valid = [s for s, ks in enumerate(idx) if ks >= 0]
        sel = w4[t][:, [idx[s] for s in valid], :]  # [q, s_valid, p]
        wt[t][:, valid, :] = sel.transpose(2, 1, 0).astype(ml_dtypes.bfloat16)

    biast = np.ascontiguousarray(
        bias.reshape(OT, P).T, dtype=np.float32
    )  # [P, OT]

    in_maps = []
    for c in range(NCORES):
        xc = x[c * MC : (c + 1) * MC, :]  # [MC, SIZE] fp32
        xt = np.ascontiguousarray(
            xc.reshape(MC, KS, P).transpose(2, 1, 0)
        ).astype(ml_dtypes.bfloat16)  # [P, KS, MC]
        in_maps.append({"xt": xt, "wt": wt, "biast": biast})

    res = run_bass_kernel_spmd(nc, in_maps, list(range(NCORES)), trace=_trace)

    out = np.empty((BATCH, SIZE), dtype=np.float32)
    for c in range(NCORES):
        o = res.results[c]["out"]  # [OT, P, MC]
        out[c * MC : (c + 1) * MC, :] = o.reshape(SIZE, MC).T
    if _trace:
        return out, res
    return out



# revision 3
# speedup vs baseline: 1.1334x; 1.1334x over previous
"""Block-sparse linear layer (x @ (mask*W).T + bias) on 8 TRN2 NeuronCores.

Strategy: data-parallel over batch rows; each core computes 1024 rows.
Mixed-precision compute: per output block-row, the latin-square mask keeps
8 of 16 k-blocks (256 cols each).  Six of those blocks run as bf16 matmuls
(K=128 subtiles, N=512) and the two blocks that fall in S={0,4,8,12} run as
fp8-e4m3 DoubleRow matmuls (K=256 per pass, 2x PE rate).  W is pre-scaled
by 64 (exact in bf16, keeps the fp8 operand out of the denormal range) and
the eviction fuses out = psum/64 + bias on the vector/scalar engines.
Offline-exact error for this split: 1.61e-2 absmax-rel (gate 2e-2).

Schedule: all W resident in SBUF (issued up front on the gpsimd queue),
x on the sync queue (fp8 x first - it is small and unblocks real fp8
matmuls that cover the startup DMA window after ~12 warmup matmuls),
out-DMAs on the sync/scalar queues.  o-tiles run in an initial 4-tile
interleaved group then pairs, chunk-major so each arriving x slab feeds
every open tile.  The final pair is tile-major with the last tile's
second half evicted split across both engines to shrink the tail.
"""

import sys
import types

import numpy as np
import ml_dtypes

BATCH = 8192
SIZE = 4096
NB = 16
BLOCK = 256
NCORES = 8
MC = BATCH // NCORES  # 1024 rows per core
P = 128
OT = SIZE // P  # 32 o-tiles
MM_N = 512
SC = 64.0  # weight pre-scale (power of two, exact in bf16)
S_FP8 = (0, 4, 8, 12)  # blocks computed in fp8 (latin-square plan)

_BUILD_CACHE = {}


def _install_ntff_hook():
    if "antenv.axon_hooks" in sys.modules:
        return
    try:
        from trn_agent_boot.trn_boot import _ntff_profile_via_ctypes

        hook = _ntff_profile_via_ctypes("/opt/axon/libaxon_pjrt.so")
        mod = types.ModuleType("antenv.axon_hooks")
        mod.get_axon_ntff_profile_hook = lambda: hook
        sys.modules["antenv.axon_hooks"] = mod
    except Exception:
        pass


def _block_keep_from_mask(mask):
    m4 = np.asarray(mask).reshape(NB, BLOCK, NB, BLOCK)
    keep = m4[:, 0, :, 0]
    uniform = np.all(m4 == keep[:, None, :, None])
    return keep if uniform else None


def _make_plan(keep):
    """Build the per-tile schedule description.

    Returns (plan, xord, ford) where plan is a hashable tuple consumed by
    _build and xord/ford give the host packing orders (bf16 block list and
    fp8 block list, in DMA issue order).
    """
    latin = np.array([[((i + j) % 16) >= 8 for j in range(NB)] for i in range(NB)])
    use_fp8 = bool(np.array_equal(keep, latin))

    # per block-row: fp8 blocks F and bf16 blocks B
    F, B = [], []
    for i in range(NB):
        kept = [j for j in range(NB) if keep[i, j]]
        if use_fp8:
            F.append([j for j in kept if j in S_FP8])
            B.append([j for j in kept if j not in S_FP8])
        else:
            F.append([])
            B.append(kept)

    # group schedule: first 4 tiles as one group, then pairs
    groups = [(0, 1, 2, 3)] + [(2 * i, 2 * i + 1) for i in range(2, NB)]

    # consumption (= DMA issue = slot) order of bf16 blocks and fp8 blocks
    xord, ford = [], []
    for ts in groups:
        for t in ts:
            for j in F[t // 2]:
                if j not in ford:
                    ford.append(j)
            for j in B[t // 2]:
                if j not in xord:
                    xord.append(j)

    n_xslot = 2 * len(xord)  # bf16 subtile slots
    n_fslot = len(ford)  # fp8 chunk slots
    NBT = max(1, max(2 * len(b) for b in B))
    NFT = max(len(f) for f in F)

    tiles_xs, tiles_fs = [], []
    for t in range(OT):
        i = t // 2
        xs = []
        for j in B[i]:
            g = xord.index(j)
            xs.extend((2 * g, 2 * g + 1))
        if not xs and not F[i]:
            xs = [0]  # fully-masked row: one zero-weight MM to clear psum
        tiles_xs.append(tuple(xs))
        tiles_fs.append(tuple(ford.index(j) for j in F[i]))

    # resident W fits comfortably for the latin plan; stream otherwise
    w_bytes_per_part = OT * (NBT * P * 2 + NFT * 2 * P)
    resident = w_bytes_per_part <= 120 * 1024

    plan = (
        n_xslot,
        n_fslot,
        tuple(tiles_fs),
        tuple(tiles_xs),
        NBT,
        NFT,
        resident,
        tuple(groups),
    )
    return plan, xord, ford


def _build(plan):
    import concourse.mybir as mybir
    import concourse.tile as tile
    from concourse import bacc

    (n_xslot, n_fslot, tiles_fs, tiles_xs, NBT, NFT, resident, groups) = plan

    bf16, f32, f8 = mybir.dt.bfloat16, mybir.dt.float32, mybir.dt.float8e4
    DR = mybir.MatmulPerfMode.DoubleRow
    IDENT = mybir.ActivationFunctionType.Identity
    MUL, ADD = mybir.AluOpType.mult, mybir.AluOpType.add
    INV = 1.0 / SC

    nc = bacc.Bacc("TRN2", target_bir_lowering=False)
    xtb_d = nc.declare_dram_parameter("xtb", [P, n_xslot, MC], bf16, isOutput=False)
    if n_fslot:
        xf8_d = nc.declare_dram_parameter("xf8", [P, n_fslot, 2, MC], f8, isOutput=False)
        wf8_d = nc.declare_dram_parameter("wf8", [OT, P, NFT, 2, P], f8, isOutput=False)
    wtb_d = nc.declare_dram_parameter("wtb", [OT, P, NBT, P], bf16, isOutput=False)
    bias_d = nc.declare_dram_parameter("biast", [P, OT], f32, isOutput=False)
    out_d = nc.declare_dram_parameter("out", [OT, P, MC], f32, isOutput=True)

    W_BUFS = OT if resident else 8

    with tile.TileContext(nc) as tc:
        with (
            tc.tile_pool(name="const", bufs=1) as const_pool,
            tc.tile_pool(name="xpool", bufs=1) as xpool,
            tc.tile_pool(name="wbpool", bufs=W_BUFS) as wbpool,
            tc.tile_pool(name="wfpool", bufs=W_BUFS) as wfpool,
            tc.tile_pool(name="opool", bufs=4) as opool,
            tc.tile_pool(name="psum", bufs=4, space="PSUM") as psum_pool,
        ):
            bias_tile = const_pool.tile([P, OT], f32)
            nc.scalar.dma_start(out=bias_tile[:], in_=bias_d[:])

            # PE warmup: keep the HAM un-throttle window alive while the
            # first x/W DMAs land (~12 cold matmuls span ~5us).
            warm = const_pool.tile([P, MM_N], bf16, name="warm")
            nc.vector.memset(warm[:], 0)
            warm_ps = psum_pool.tile([P, MM_N], f32, name="warm_ps", tag="ps")
            N_WARM = 12
            for i in range(N_WARM):
                nc.tensor.matmul(
                    warm_ps[:],
                    lhsT=warm[:, 0:P],
                    rhs=warm[:],
                    start=(i == 0),
                    stop=(i == N_WARM - 1),
                )

            # ---- DMA issue ----
            # fp8 x first on sync (small, unblocks the fp8 matmuls), then
            # bf16 x groups in consumption order.
            f_tile = None
            if n_fslot:
                f_tile = xpool.tile([P, n_fslot, 2, MC], f8, name="xf8t", uniquify=False)
                half = (n_fslot + 1) // 2
                nc.sync.dma_start(out=f_tile[:, 0:half], in_=xf8_d[:, 0:half])
                if half < n_fslot:
                    nc.sync.dma_start(out=f_tile[:, half:], in_=xf8_d[:, half:])

            x_tiles = []
            for g in range(n_xslot // 2):
                xg = xpool.tile([P, 2, MC], bf16, name=f"x_g{g}", uniquify=False)
                nc.sync.dma_start(out=xg[:], in_=xtb_d[:, 2 * g : 2 * g + 2, :])
                x_tiles.append(xg)

            def x_ap(slot):
                return x_tiles[slot // 2][:, slot % 2, :]

            # W on the gpsimd queue.
            wtb_tiles, wf8_tiles = {}, {}

            def w_dma(t):
                if n_fslot and NFT:
                    wf = wfpool.tile([P, NFT, 2, P], f8, name="wf8_tile")
                    nc.gpsimd.dma_start(out=wf[:], in_=wf8_d[t])
                    wf8_tiles[t] = wf
                wb = wbpool.tile([P, NBT, P], bf16, name="wtb_tile")
                nc.gpsimd.dma_start(out=wb[:], in_=wtb_d[t])
                wtb_tiles[t] = wb

            if resident:
                if n_fslot:
                    # fp8 W for the first group leads so fp8 MMs can start
                    for t in range(4):
                        wf = wfpool.tile([P, NFT, 2, P], f8, name="wf8_tile")
                        nc.gpsimd.dma_start(out=wf[:], in_=wf8_d[t])
                        wf8_tiles[t] = wf
                for t in range(4):
                    wb = wbpool.tile([P, NBT, P], bf16, name="wtb_tile")
                    nc.gpsimd.dma_start(out=wb[:], in_=wtb_d[t])
                    wtb_tiles[t] = wb
                for t in range(4, OT):
                    w_dma(t)
            else:
                for t in range(4):
                    w_dma(t)

            # ---- compute emission ----
            def units(t):
                return [("f", fi) for fi in range(len(tiles_fs[t]))] + [
                    ("x", s) for s in tiles_xs[t]
                ]

            def emit_unit(t, kind, arg, ps_t, n_done, h_list=(0, 1)):
                """One accumulation unit = 1 or 2 matmuls (per half)."""
                total = len(units(t))
                first = n_done[t] == 0
                n_done[t] += 1
                last = n_done[t] == total
                for h in h_list:
                    sl = slice(h * MM_N, (h + 1) * MM_N)
                    if kind == "f":
                        cs = tiles_fs[t][arg]
                        nc.tensor.matmul(
                            ps_t[:, sl],
                            lhsT=wf8_tiles[t][:, arg, :, :],
                            rhs=f_tile[:, cs, :, sl],
                            start=first,
                            stop=last,
                            perf_mode=DR,
                        )
                    else:
                        u = tiles_xs[t].index(arg)
                        nc.tensor.matmul(
                            ps_t[:, sl],
                            lhsT=wtb_tiles[t][:, u, :],
                            rhs=x_ap(arg)[:, sl],
                            start=first,
                            stop=last,
                        )

            def evict(t, ps_t):
                o = opool.tile([P, MC], f32, name="o_tile")
                for h in (0, 1):
                    sl = slice(h * MM_N, (h + 1) * MM_N)
                    if t % 2 == 0:
                        nc.vector.tensor_scalar(
                            o[:, sl], ps_t[:, sl], INV, bias_tile[:, t : t + 1], MUL, ADD
                        )
                        nc.sync.dma_start(out=out_d[t, :, sl], in_=o[:, sl])
                    else:
                        nc.scalar.activation(
                            o[:, sl], ps_t[:, sl], IDENT,
                            bias=bias_tile[:, t : t + 1], scale=INV,
                        )
                        nc.scalar.dma_start(out=out_d[t, :, sl], in_=o[:, sl])

            def emit_group(ts):
                """Interleaved (chunk-major) emission for tiles ts."""
                ps = {t: psum_pool.tile([P, MC], f32, name="ps", tag="ps") for t in ts}
                n_done = {t: 0 for t in ts}
                max_f = max((len(tiles_fs[t]) for t in ts), default=0)
                for fi in range(max_f):
                    for t in ts:
                        if fi < len(tiles_fs[t]):
                            emit_unit(t, "f", fi, ps[t], n_done)
                for s in range(n_xslot):
                    for t in ts:
                        if s in tiles_xs[t]:
                            emit_unit(t, "x", s, ps[t], n_done)
                            if n_done[t] == len(units(t)):
                                evict(t, ps[t])
                for t in ts:  # tiles that ended on an fp8 unit (none normally)
                    if n_done[t] == len(units(t)) and len(tiles_xs[t]) == 0:
                        evict(t, ps[t])

            def emit_tail(ts):
                """Tile-major final group; last tile h-major with split evict."""
                ps = {t: psum_pool.tile([P, MC], f32, name="ps", tag="ps") for t in ts}
                n_done = {t: 0 for t in ts}
                for t in ts[:-1]:
                    for kind, arg in units(t):
                        emit_unit(t, kind, arg, ps[t], n_done)
                    evict(t, ps[t])
                t = ts[-1]
                ul = units(t)
                o = opool.tile([P, MC], f32, name="o_tile")
                for h in (0, 1):
                    n_done[t] = 0  # per-half accumulation group flags
                    for kind, arg in ul:
                        emit_unit(t, kind, arg, ps[t], n_done, h_list=(h,))
                    if h == 0:
                        # evict half 0 while half 1 accumulates
                        sl = slice(0, MM_N)
                        nc.scalar.activation(
                            o[:, sl], ps[t][:, sl], IDENT,
                            bias=bias_tile[:, t : t + 1], scale=INV,
                        )
                        nc.scalar.dma_start(out=out_d[t, :, sl], in_=o[:, sl])
                # final half split across both engines
                q = MM_N // 2
                sl_v = slice(MM_N, MM_N + q)
                sl_s = slice(MM_N + q, 2 * MM_N)
                nc.vector.tensor_scalar(
                    o[:, sl_v], ps[t][:, sl_v], INV, bias_tile[:, t : t + 1], MUL, ADD
                )
                nc.sync.dma_start(out=out_d[t, :, sl_v], in_=o[:, sl_v])
                nc.scalar.activation(
                    o[:, sl_s], ps[t][:, sl_s], IDENT,
                    bias=bias_tile[:, t : t + 1], scale=INV,
                )
                nc.scalar.dma_start(out=out_d[t, :, sl_s], in_=o[:, sl_s])

            for gi, ts in enumerate(groups):
                if not resident:
                    for t in ts:
                        if t not in wtb_tiles:
                            w_dma(t)
                if gi == len(groups) - 1 and len(ts) >= 2:
                    emit_tail(ts)
                else:
                    emit_group(ts)
    nc.compile()
    return nc


def _get_kernel(plan):
    if plan not in _BUILD_CACHE:
        _BUILD_CACHE[plan] = _build(plan)
    return _BUILD_CACHE[plan]


def kernel(x, weight, bias, mask, _trace=False):
    import concourse.mybir as mybir
    from concourse.bass_utils import run_bass_kernel_spmd

    _install_ntff_hook()

    f8_np = mybir.dt.np(mybir.dt.float8e4)
    bf16_np = ml_dtypes.bfloat16

    x = np.asarray(x)
    weight = np.asarray(weight)
    bias = np.asarray(bias, dtype=np.float32)
    keep = _block_keep_from_mask(mask)
    if keep is None:
        weight = np.where(np.asarray(mask), weight, 0.0).astype(np.float32)
        keep = np.ones((NB, NB), dtype=bool)
    plan, xord, ford = _make_plan(keep)
    (n_xslot, n_fslot, tiles_fs, tiles_xs, NBT, NFT, _, _) = plan

    nc = _get_kernel(plan)

    ws = (weight * SC).astype(np.float32)
    w4 = ws.reshape(OT, P, SIZE)  # [t, q, k]

    # bf16 W: wtb[t, p, u, q] = SC*W[t*P+q, slot_k(u)*P + p]
    wtb = np.zeros((OT, P, NBT, P), dtype=bf16_np)
    for t in range(OT):
        if not keep[t // 2].any():
            continue  # zero-pad (fully masked row)
        xs = tiles_xs[t]
        # map slot -> absolute subtile: slot 2g+e of block xord[g]
        subs = [2 * xord[s // 2] + (s % 2) for s in xs]
        sel = w4[t].reshape(P, SIZE // P, P)[:, subs, :]  # [q, u, p]
        wtb[t][:, : len(xs), :] = sel.transpose(2, 1, 0).astype(bf16_np)

    # fp8 W: wf8[t, p, f, e, q] = e4m3(SC*W[t*P+q, ford[fs[f]]*BLOCK + e*P + p])
    wf8 = None
    if n_fslot:
        wf8 = np.zeros((OT, P, NFT, 2, P), dtype=f8_np)
        for t in range(OT):
            for f, cs in enumerate(tiles_fs[t]):
                j = ford[cs]
                blk = w4[t][:, j * BLOCK : (j + 1) * BLOCK]  # [q, 256]
                blk = blk.reshape(P, 2, P)  # [q, e, p]
                wf8[t][:, f, :, :] = blk.transpose(2, 1, 0).astype(f8_np)

    biast = np.ascontiguousarray(bias.reshape(OT, P).T, dtype=np.float32)

    # x packing per core
    xsubs = []
    for g in range(n_xslot // 2):
        xsubs.extend((2 * xord[g], 2 * xord[g] + 1))

    in_maps = []
    for c in range(NCORES):
        xc = x[c * MC : (c + 1) * MC, :]  # [MC, SIZE]
        x3 = xc.reshape(MC, SIZE // P, P)  # [m, sub, p]
        xtb = np.ascontiguousarray(
            x3[:, xsubs, :].transpose(2, 1, 0)
        ).astype(bf16_np)  # [P, n_xslot, MC]
        im = {"xtb": xtb, "wtb": wtb, "biast": biast}
        if n_fslot:
            xf = np.empty((P, n_fslot, 2, MC), dtype=f8_np)
            for cslot, j in enumerate(ford):
                blk = x3[:, 2 * j : 2 * j + 2, :].astype(f8_np)  # [m, e, p]
                xf[:, cslot] = blk.transpose(2, 1, 0)
            im["xf8"] = xf
            im["wf8"] = wf8
        in_maps.append(im)

    res = run_bass_kernel_spmd(nc, in_maps, list(range(NCORES)), trace=_trace)

    out = np.empty((BATCH, SIZE), dtype=np.float32)
    for c in range(NCORES):
        o = res.results[c]["out"]  # [OT, P, MC]
        out[c * MC : (c + 1) * MC, :] = o.reshape(SIZE, MC).T
    if _trace:
        return out, res
    return out
